# revision 66
# baseline (speedup 1.0000x reference)
import sys

sys.path.insert(0, "/opt/trn_rl_repo")
import numpy as np

S, B, D, H = 1024, 4, 1024, 16
DH = D // H  # 64
HPC = 8  # heads per core
PG = HPC * DH  # 512 proj dims per core
N_CORES = 8
ATT_SCALE = 1.0 / np.sqrt(DH)
NT = S // 128  # 8 t-chunks
NST = 2  # s halves
NSB = 4  # 128-row s blocks per half

_prog_cache = {}


def _build_program(kc=8):
    import concourse.tile as tile
    from concourse import bacc, mybir

    nc = bacc.Bacc(
        "TRN2",
        target_bir_lowering=False,
        debug=False,
        enable_asserts=False,
        num_devices=N_CORES,
    )
    f32 = mybir.dt.float32
    f16 = mybir.dt.float16
    Exp = mybir.ActivationFunctionType.Exp

    kpad = kc * 128
    xq = nc.dram_tensor("xq", (kpad, S), f16, kind="ExternalInput").ap()
    xk = nc.dram_tensor("xk", (kpad, S), f16, kind="ExternalInput").ap()
    xv = nc.dram_tensor("xv", (kpad, S), f16, kind="ExternalInput").ap()
    wq = nc.dram_tensor("wq", (kpad, PG), f16, kind="ExternalInput").ap()
    wk = nc.dram_tensor("wk", (kpad, PG), f16, kind="ExternalInput").ap()
    wv = nc.dram_tensor("wv", (kpad, PG), f16, kind="ExternalInput").ap()
    wo = nc.dram_tensor("wo", (PG, D), f16, kind="ExternalInput").ap()
    out = nc.dram_tensor("out", (S, D), f16, kind="ExternalOutput").ap()

    with tile.TileContext(nc) as tc:
        import contextlib

        with contextlib.ExitStack() as ctx:
            persist = ctx.enter_context(tc.tile_pool(name="persist", bufs=1))
            xqt = persist.tile([128, kc * S], f16, tag="xqt")
            xkt = persist.tile([128, kc * S], f16, tag="xkt")
            xvt = persist.tile([128, kc * S], f16, tag="xvt")
            wqt = persist.tile([128, kc * PG], f16, tag="wqt")
            wkt = persist.tile([128, kc * PG], f16, tag="wkt")
            wvt = persist.tile([128, kc * PG], f16, tag="wvt")
            wot = persist.tile([128, 4 * D], f16, tag="wot")
            # projT: chunk c at free c*S holds heads 2c (p0-63), 2c+1 (p64-127)
            qproj = persist.tile([128, 4 * S], f16, tag="qproj")
            kproj = persist.tile([128, 4 * S], f16, tag="kproj")
            # v natural + ones col: t-chunk tc at free tc*(PG+HPC), head h at +h*65
            vaug = persist.tile([128, NT * (PG + HPC)], f16, tag="vaug")
            # ctx^T: pg-chunk j at free j*S
            ctxT = persist.tile([128, 4 * S], f16, tag="ctxT")

            nc.gpsimd.memset(vaug[:], 1.0)
            # PE p-state warmup: the tensor engine ramps to full clock only
            # after ~3us of continuous work; throwaway matmuls during the
            # DMA-gated startup window let real projections start at speed.
            wrm = persist.tile([128, 512], f16, tag="wrm")
            nc.vector.memset(wrm[:], 0.5)

            # ---- input loads ----
            # Act HWDGE queue. Column-half weight loads so the first head
            # pair's k-proj gate (wq-h0 + xq + wk-h0 + xk) lands earliest.
            def w_half(dst, src, half):
                nc.scalar.dma_start(
                    dst[:].rearrange("p (k n) -> p k n", n=PG)[
                        :, :, half * 256 : (half + 1) * 256
                    ],
                    src.rearrange("(k p) n -> p k n", p=128)[
                        :, :, half * 256 : (half + 1) * 256
                    ],
                )

            w_half(wqt, wq, 0)
            w_half(wkt, wk, 0)
            # The first exp gates on just the st0 column-halves of q-proj
            # and k-proj; later halves stream in behind the Act cadence.
            def x_half(dst, src, kk, half):
                nc.scalar.dma_start(
                    dst[:, kk * S + half * 512 : kk * S + half * 512 + 512],
                    src[kk * 128 : (kk + 1) * 128, half * 512 : half * 512 + 512],
                )

            for kk in range(kc):
                x_half(xqt, xq, kk, 0)
                nc.scalar.dma_start(
                    xkt[:, kk * S : (kk + 1) * S], xk[kk * 128 : (kk + 1) * 128, :]
                )
            w_half(wqt, wq, 1)
            w_half(wkt, wk, 1)
            for kk in range(kc):
                x_half(xqt, xq, kk, 1)
            # SP queue, delayed so it doesn't steal DMA bandwidth from the gate
            with tc.tile_wait_until(0.022):
                nc.sync.dma_start(
                    wvt[:].rearrange("p (k n) -> p k n", n=PG),
                    wv.rearrange("(k p) n -> p k n", p=128),
                )
                for kk in range(kc):
                    nc.sync.dma_start(
                        xvt[:, kk * S : (kk + 1) * S], xv[kk * 128 : (kk + 1) * 128, :]
                    )
            with tc.tile_wait_until(0.042):
                nc.sync.dma_start(
                    wot[:].rearrange("p (c n) -> p c n", c=4),
                    wo.rearrange("(c p) n -> p c n", p=128),
                )

            pap = ctx.enter_context(tc.tile_pool(name="pap", bufs=2, space="PSUM"))
            scp = ctx.enter_context(tc.tile_pool(name="scp", bufs=2, space="PSUM"))
            pvp = ctx.enter_context(tc.tile_pool(name="pvp", bufs=2, space="PSUM"))

            wps = scp.tile([128, 1024], f32, tag="ps", name="warm_ps")
            for wi in range(10):
                nc.tensor.matmul(
                    wps[:, 0:512], wrm[:, 0:128], wrm[:], start=True, stop=True
                )
            dvep = ctx.enter_context(tc.tile_pool(name="dvep", bufs=4))
            expp = ctx.enter_context(tc.tile_pool(name="expp", bufs=38 if kc == 8 else 26))
            ctxp = ctx.enter_context(tc.tile_pool(name="ctxp", bufs=8))
            recp = ctx.enter_context(tc.tile_pool(name="recp", bufs=4))

            def proj_half(xt, wt, dst, c, st, nm, pool=None, tg="pa"):
                pool = pool or pap
                acc = pool.tile([128, 512], f32, tag=tg, name=f"acc_{nm}{c}_{st}")
                for kk in range(kc):
                    nc.tensor.matmul(
                        acc[:],
                        wt[:, kk * PG + c * 128 : kk * PG + (c + 1) * 128],
                        xt[:, kk * S + st * 512 : kk * S + st * 512 + 512],
                        start=(kk == 0),
                        stop=(kk == kc - 1),
                    )
                nc.vector.tensor_copy(
                    dst[:, c * S + st * 512 : c * S + st * 512 + 512], acc[:]
                )

            def v_proj_chunk(tc_):
                acc = pap.tile([128, 512], f32, tag="pa", name=f"acc_v{tc_}")
                for kk in range(kc):
                    nc.tensor.matmul(
                        acc[:],
                        xvt[:, kk * S + tc_ * 128 : kk * S + (tc_ + 1) * 128],
                        wvt[:, kk * PG : (kk + 1) * PG],
                        start=(kk == 0),
                        stop=(kk == kc - 1),
                    )
                vslice = vaug[:, tc_ * (PG + HPC) : (tc_ + 1) * (PG + HPC)]
                nc.vector.tensor_copy(
                    vslice.rearrange("p (h e) -> p h e", e=DH + 1)[:, :, 0:DH],
                    acc[:].rearrange("p (h e) -> p h e", h=HPC),
                )

            ctxn = {}
            for st in range(NST):
                for sb in range(NSB):
                    ctxn[(st, sb)] = ctxp.tile(
                        [128, PG], f16, tag="ctxn", name=f"ctxn_{st}_{sb}"
                    )

            ex_tiles = {}

            def scores_block(st, h):
                # two t-chunks share one 2-bank psum tile so a single wide
                # activation (1024 free) amortizes the per-instruction access
                # bubble on the Activation engine
                c, po = h // 2, (h % 2) * 64
                fo = c * S
                exs = []
                for tb in range(NT // 2):
                    sc_ps = scp.tile(
                        [128, 1024], f32, tag="ps", name=f"sc_{st}_{h}_{tb}"
                    )
                    for half in range(2):
                        tch = tb * 2 + half
                        nc.tensor.matmul(
                            sc_ps[:, half * 512 : half * 512 + 512],
                            kproj[po : po + 64, fo + tch * 128 : fo + (tch + 1) * 128],
                            qproj[po : po + 64, fo + st * 512 : fo + st * 512 + 512],
                            start=True,
                            stop=True,
                        )
                    ex = expp.tile(
                        [128, 1024], f16, tag="exp", name=f"ex_{st}_{h}_{tb}"
                    )
                    nc.scalar.activation(ex[:], sc_ps[:], Exp)
                    exs.append(ex)
                ex_tiles[(st, h)] = exs

            # pv psum slots: pvp; in st1 alternate with pap slots (projection
            # accumulators are done by then, and pap tiles are allocated
            # before these in pool-rotation order)
            pv_pool_of = {}
            for _h in range(HPC):
                pv_pool_of[(0, _h)] = pap if _h % 2 == 0 else pvp
                pv_pool_of[(1, _h)] = pvp

            _pool_tag = {"pvp": "pv", "pap": "pa", "oap": "oa"}

            def pv_block(st, h):
                pool = pv_pool_of[(st, h)]
                pv = pool.tile(
                    [128, 512], f32, tag=_pool_tag[pool.name], name=f"pv_{st}_{h}"
                )
                exs = ex_tiles.pop((st, h))
                # tch-outer so each ex tile is fully consumed (all 4 sb groups)
                # as soon as its vaug chunk lands -> frees exp-pool slots early.
                # PSUM start zeroes the whole 2KB zero-region lazily, so only
                # the very first matmul starts; the other sb groups begin from
                # the pending-zero state.
                for tch in range(NT):
                    for sb in range(NSB):
                        nc.tensor.matmul(
                            pv[:, sb * 128 : sb * 128 + 65],
                            exs[tch // 2][
                                :, (tch % 2) * 512 + sb * 128 : (tch % 2) * 512 + (sb + 1) * 128
                            ],
                            vaug[
                                :,
                                tch * (PG + HPC) + h * (DH + 1) : tch * (PG + HPC)
                                + (h + 1) * (DH + 1),
                            ],
                            start=(tch == 0 and sb == 0),
                            stop=(tch == NT - 1 and sb == NSB - 1),
                            skip_group_check=True,
                        )
                rec4 = recp.tile([128, 4], f32, tag="rec", name=f"rec_{st}_{h}")
                nc.vector.reciprocal(
                    rec4[:].rearrange("p (g e) -> p g e", e=1),
                    pv[:].rearrange("p (g e) -> p g e", e=128)[:, :, 64:65],
                )
                for sb in range(NSB):
                    nc.vector.tensor_scalar(
                        ctxn[(st, sb)][:, h * DH : (h + 1) * DH],
                        pv[:, sb * 128 : sb * 128 + DH],
                        rec4[:, sb : sb + 1],
                        None,
                        mybir.AluOpType.mult,
                    )
                if h % 2 == 1:
                    j = h // 2
                    for sb in range(NSB):
                        nc.sync.dma_start_transpose(
                            ctxT[
                                :,
                                j * S + st * 512 + sb * 128 : j * S
                                + st * 512
                                + (sb + 1) * 128,
                            ],
                            ctxn[(st, sb)][:, j * 128 : (j + 1) * 128],
                        )

            def outproj(st, sbs=None):
                for sb in sbs if sbs is not None else range(NSB):
                    osb = dvep.tile([128, D], f16, tag="osb", name=f"osb_{st}_{sb}")
                    for nt in range(2):
                        gi = sb * 2 + nt
                        # st1 tail: scores pool is drained by then, borrow its
                        # banks so more groups pre-accumulate in parallel
                        pool, tg = (pap, "pa")
                        if st == 1 and sb % 2 == 1:
                            # round-2 groups: pvp slots free right after the
                            # tail transposes, ahead of pap/scp round-1 evicts
                            pool, tg = (pvp, "pv")
                        elif st == 1 and sb == 2:
                            pool, tg = (scp, "ps")
                        acc = pool.tile(
                            [128, 512], f32, tag=tg, name=f"oacc_{st}_{sb}_{nt}"
                        )
                        for j in range(4):
                            nc.tensor.matmul(
                                acc[:],
                                ctxT[
                                    :,
                                    j * S + st * 512 + sb * 128 : j * S
                                    + st * 512
                                    + (sb + 1) * 128,
                                ],
                                wot[:, j * D + nt * 512 : j * D + nt * 512 + 512],
                                start=(j == 0),
                                stop=(j == 3),
                            )
                        # tail evictions split across DVE and the by-then-idle
                        # Act engine so the last groups drain in parallel
                        if st == 1 and nt == 1:
                            nc.scalar.copy(osb[:, nt * 512 : nt * 512 + 512], acc[:])
                        else:
                            nc.vector.tensor_copy(
                                osb[:, nt * 512 : nt * 512 + 512], acc[:]
                            )
                    nc.sync.dma_start(
                        out[st * 512 + sb * 128 : st * 512 + (sb + 1) * 128, :], osb[:]
                    )

            # q0/q1 pre-run while xk loads; k0 immediately at the xk gate so
            # the exp stream starts earliest. q2/q3 fold into the head stream.
            # PV deferred one head behind its scores so PE never head-of-line
            # blocks on the exp stream.
            # Emission order = program order: every consumer AFTER its
            # producers. kproj needs BOTH free-halves per pair before that
            # pair's scores (free dim = key positions); qproj's halves map to
            # the query split so st1 halves defer. pv blocks must follow the
            # v chunks they read; the scheduler still streams their matmuls
            # as each vaug chunk lands (subtile deps).
            proj_half(xqt, wqt, qproj, 0, 0, "q")
            proj_half(xkt, wkt, kproj, 0, 0, "k")
            proj_half(xkt, wkt, kproj, 0, 1, "k", pool=pvp, tg="pv")
            scores_block(0, 0)
            scores_block(0, 1)
            proj_half(xqt, wqt, qproj, 1, 0, "q")
            proj_half(xkt, wkt, kproj, 1, 0, "k")
            proj_half(xkt, wkt, kproj, 1, 1, "k")
            scores_block(0, 2)
            scores_block(0, 3)
            proj_half(xqt, wqt, qproj, 2, 0, "q")
            proj_half(xkt, wkt, kproj, 2, 0, "k")
            proj_half(xkt, wkt, kproj, 2, 1, "k")
            scores_block(0, 4)
            scores_block(0, 5)
            proj_half(xqt, wqt, qproj, 3, 0, "q")
            proj_half(xkt, wkt, kproj, 3, 0, "k")
            proj_half(xkt, wkt, kproj, 3, 1, "k")
            scores_block(0, 6)
            scores_block(0, 7)
            proj_half(xqt, wqt, qproj, 0, 1, "q")
            scores_block(1, 0)
            scores_block(1, 1)
            for tc_ in range(NT):
                v_proj_chunk(tc_)
            pv_block(0, 0)
            pv_block(0, 1)
            proj_half(xqt, wqt, qproj, 1, 1, "q")
            for _h in range(2, HPC):
                pv_block(0, _h)
            scores_block(1, 2)
            scores_block(1, 3)
            pv_block(1, 0)
            pv_block(1, 1)
            outproj(0, (0, 1))
            proj_half(xqt, wqt, qproj, 2, 1, "q")
            scores_block(1, 4)
            scores_block(1, 5)
            pv_block(1, 2)
            pv_block(1, 3)
            outproj(0, (2, 3))
            proj_half(xqt, wqt, qproj, 3, 1, "q")
            scores_block(1, 6)
            scores_block(1, 7)
            pv_block(1, 4)
            pv_block(1, 5)
            pv_block(1, 6)
            pv_block(1, 7)
            # emitted last = lowest priority: pure PE-gap fillers, scheduled
            # whenever their ctxT pair-chunks are ready
            outproj(1)

    nc.compile()
    return nc


def _get_program(kc=8):
    if kc not in _prog_cache:
        _prog_cache[kc] = _build_program(kc)
    return _prog_cache[kc]


def _pad_k(a, kc):
    """(1024, n) f16 -> (kc*128, n); row 1024 = bias slot (set by caller) when kc=9."""
    if kc == 8:
        return np.ascontiguousarray(a, np.float16)
    p = np.zeros((kc * 128, a.shape[1]), np.float16)
    p[:D] = a
    return p


def _np_reference(q, k, v, attn_mask, Wq, bq, Wk, bk, Wv, bv, Wo, bo):
    def split_heads(x):
        return x.reshape(S, B, H, DH).transpose(2, 1, 0, 3)

    qh = split_heads(q @ Wq.T + bq)
    kh = split_heads(k @ Wk.T + bk)
    vh = split_heads(v @ Wv.T + bv)
    scores = np.einsum("hbsd,hbtd->hbst", qh, kh) * ATT_SCALE + attn_mask
    m = scores.max(-1, keepdims=True)
    e = np.exp(scores - m)
    probs = e / e.sum(-1, keepdims=True)
    ctx = np.einsum("hbst,hbtd->hbsd", probs, vh)
    ctx = ctx.transpose(2, 1, 0, 3).reshape(S, B, D)
    return (ctx @ Wo.T + bo).astype(np.float32)


def kernel(q, k, v, attn_mask, Wq, bq, Wk, bk, Wv, bv, Wo, bo, _want_results=False, _trace=False):
    q, k, v = (np.asarray(x, np.float32) for x in (q, k, v))
    attn_mask = np.asarray(attn_mask, np.float32)
    Wq, bq, Wk, bk, Wv, bv, Wo, bo = (
        np.asarray(x, np.float32) for x in (Wq, bq, Wk, bk, Wv, bv, Wo, bo)
    )
    if attn_mask.any():
        return _np_reference(q, k, v, attn_mask, Wq, bq, Wk, bk, Wv, bv, Wo, bo)

    from concourse import bass_utils

    zero_bias = not (bq.any() or bk.any() or bv.any())
    kc = 8 if zero_bias else 9
    nc = _get_program(kc)

    xT = {}
    for b in range(B):
        for nm, t in (("q", q), ("k", k), ("v", v)):
            a = _pad_k(t[:, b, :].T.astype(np.float16), kc)
            if kc > 8:
                a[D] = 1.0  # bias row
            xT[(nm, b)] = a

    in_maps = []
    for cid in range(N_CORES):
        b, g = cid >> 1, cid & 1
        cols = slice(g * PG, (g + 1) * PG)
        wqT = _pad_k((Wq[cols].T * ATT_SCALE).astype(np.float16), kc)
        wkT = _pad_k(Wk[cols].T.astype(np.float16), kc)
        wvT = _pad_k(Wv[cols].T.astype(np.float16), kc)
        if kc > 8:
            wqT[D] = (bq[cols] * ATT_SCALE).astype(np.float16)
            wkT[D] = bk[cols].astype(np.float16)
            wvT[D] = bv[cols].astype(np.float16)
        woT = np.ascontiguousarray(Wo[:, cols].T.astype(np.float16))
        in_maps.append(
            {
                "xq": xT[("q", b)],
                "xk": xT[("k", b)],
                "xv": xT[("v", b)],
                "wq": wqT,
                "wk": wkT,
                "wv": wvT,
                "wo": woT,
                "ident": np.eye(128, dtype=np.float16),
            }
        )

    import tempfile

    kw = {}
    if _trace:
        kw = dict(trace=True, tmpdir=tempfile.mkdtemp(prefix="bass_trace_"))
    res = bass_utils.run_bass_kernel_spmd(nc, in_maps, core_ids=list(range(N_CORES)), **kw)
    out = np.empty((S, B, D), np.float32)
    for b in range(B):
        out[:, b, :] = (
            res.results[2 * b]["out"].astype(np.float32)
            + res.results[2 * b + 1]["out"].astype(np.float32)
            + bo
        )
    if _want_results:
        return out, res
    return out


# revision 67
# speedup vs baseline: 1.0046x; 1.0046x over previous
import sys

sys.path.insert(0, "/opt/trn_rl_repo")
import numpy as np

S, B, D, H = 1024, 4, 1024, 16
DH = D // H  # 64
HPC = 8  # heads per core
PG = HPC * DH  # 512 proj dims per core
N_CORES = 8
ATT_SCALE = 1.0 / np.sqrt(DH)
NT = S // 128  # 8 t-chunks
NST = 2  # s halves
NSB = 4  # 128-row s blocks per half

_prog_cache = {}


def _build_program(kc=8):
    import concourse.tile as tile
    from concourse import bacc, mybir

    nc = bacc.Bacc(
        "TRN2",
        target_bir_lowering=False,
        debug=False,
        enable_asserts=False,
        num_devices=N_CORES,
    )
    f32 = mybir.dt.float32
    f16 = mybir.dt.float16
    Exp = mybir.ActivationFunctionType.Exp

    kpad = kc * 128
    xq = nc.dram_tensor("xq", (kpad, S), f16, kind="ExternalInput").ap()
    xk = nc.dram_tensor("xk", (kpad, S), f16, kind="ExternalInput").ap()
    xv = nc.dram_tensor("xv", (kpad, S), f16, kind="ExternalInput").ap()
    wq = nc.dram_tensor("wq", (kpad, PG), f16, kind="ExternalInput").ap()
    wk = nc.dram_tensor("wk", (kpad, PG), f16, kind="ExternalInput").ap()
    wv = nc.dram_tensor("wv", (kpad, PG), f16, kind="ExternalInput").ap()
    wo = nc.dram_tensor("wo", (PG, D), f16, kind="ExternalInput").ap()
    out = nc.dram_tensor("out", (S, D), f16, kind="ExternalOutput").ap()

    with tile.TileContext(nc) as tc:
        import contextlib

        with contextlib.ExitStack() as ctx:
            persist = ctx.enter_context(tc.tile_pool(name="persist", bufs=1))
            xqt = persist.tile([128, kc * S], f16, tag="xqt")
            xkt = persist.tile([128, kc * S], f16, tag="xkt")
            xvt = persist.tile([128, kc * S], f16, tag="xvt")
            wqt = persist.tile([128, kc * PG], f16, tag="wqt")
            wkt = persist.tile([128, kc * PG], f16, tag="wkt")
            wvt = persist.tile([128, kc * PG], f16, tag="wvt")
            wot = persist.tile([128, 4 * D], f16, tag="wot")
            # projT: chunk c at free c*S holds heads 2c (p0-63), 2c+1 (p64-127)
            qproj = persist.tile([128, 4 * S], f16, tag="qproj")
            kproj = persist.tile([128, 4 * S], f16, tag="kproj")
            # v natural + ones col: t-chunk tc at free tc*(PG+HPC), head h at +h*65
            vaug = persist.tile([128, NT * (PG + HPC)], f16, tag="vaug")
            # ctx^T: pg-chunk j at free j*S
            ctxT = persist.tile([128, 4 * S], f16, tag="ctxT")

            nc.gpsimd.memset(vaug[:], 1.0)
            # PE p-state warmup: the tensor engine ramps to full clock only
            # after ~3us of continuous work; throwaway matmuls during the
            # DMA-gated startup window let real projections start at speed.
            wrm = persist.tile([128, 512], f16, tag="wrm")
            nc.vector.memset(wrm[:], 0.5)

            # ---- input loads ----
            # Act HWDGE queue. Column-half weight loads so the first head
            # pair's k-proj gate (wq-h0 + xq + wk-h0 + xk) lands earliest.
            def w_half(dst, src, half):
                nc.scalar.dma_start(
                    dst[:].rearrange("p (k n) -> p k n", n=PG)[
                        :, :, half * 256 : (half + 1) * 256
                    ],
                    src.rearrange("(k p) n -> p k n", p=128)[
                        :, :, half * 256 : (half + 1) * 256
                    ],
                )

            w_half(wqt, wq, 0)
            w_half(wkt, wk, 0)
            # The first exp gates on just the st0 column-halves of q-proj
            # and k-proj; later halves stream in behind the Act cadence.
            def x_half(dst, src, kk, half):
                nc.scalar.dma_start(
                    dst[:, kk * S + half * 512 : kk * S + half * 512 + 512],
                    src[kk * 128 : (kk + 1) * 128, half * 512 : half * 512 + 512],
                )

            for kk in range(kc):
                x_half(xqt, xq, kk, 0)
                nc.scalar.dma_start(
                    xkt[:, kk * S : (kk + 1) * S], xk[kk * 128 : (kk + 1) * 128, :]
                )
            w_half(wqt, wq, 1)
            w_half(wkt, wk, 1)
            for kk in range(kc):
                x_half(xqt, xq, kk, 1)
            # SP queue, delayed so it doesn't steal DMA bandwidth from the gate
            with tc.tile_wait_until(0.022):
                nc.sync.dma_start(
                    wvt[:].rearrange("p (k n) -> p k n", n=PG),
                    wv.rearrange("(k p) n -> p k n", p=128),
                )
                for kk in range(kc):
                    nc.sync.dma_start(
                        xvt[:, kk * S : (kk + 1) * S], xv[kk * 128 : (kk + 1) * 128, :]
                    )
            with tc.tile_wait_until(0.042):
                nc.sync.dma_start(
                    wot[:].rearrange("p (c n) -> p c n", c=4),
                    wo.rearrange("(c p) n -> p c n", p=128),
                )

            pap = ctx.enter_context(tc.tile_pool(name="pap", bufs=2, space="PSUM"))
            scp = ctx.enter_context(tc.tile_pool(name="scp", bufs=2, space="PSUM"))
            pvp = ctx.enter_context(tc.tile_pool(name="pvp", bufs=2, space="PSUM"))

            wps = scp.tile([128, 1024], f32, tag="ps", name="warm_ps")
            for wi in range(10):
                nc.tensor.matmul(
                    wps[:, 0:512], wrm[:, 0:128], wrm[:], start=True, stop=True
                )
            dvep = ctx.enter_context(tc.tile_pool(name="dvep", bufs=4))
            expp = ctx.enter_context(tc.tile_pool(name="expp", bufs=38 if kc == 8 else 26))
            ctxp = ctx.enter_context(tc.tile_pool(name="ctxp", bufs=8))
            recp = ctx.enter_context(tc.tile_pool(name="recp", bufs=4))

            def proj_half(xt, wt, dst, c, st, nm, pool=None, tg="pa"):
                pool = pool or pap
                acc = pool.tile([128, 512], f32, tag=tg, name=f"acc_{nm}{c}_{st}")
                for kk in range(kc):
                    nc.tensor.matmul(
                        acc[:],
                        wt[:, kk * PG + c * 128 : kk * PG + (c + 1) * 128],
                        xt[:, kk * S + st * 512 : kk * S + st * 512 + 512],
                        start=(kk == 0),
                        stop=(kk == kc - 1),
                    )
                nc.vector.tensor_copy(
                    dst[:, c * S + st * 512 : c * S + st * 512 + 512], acc[:]
                )

            def v_proj_chunk(tc_):
                acc = pap.tile([128, 512], f32, tag="pa", name=f"acc_v{tc_}")
                for kk in range(kc):
                    nc.tensor.matmul(
                        acc[:],
                        xvt[:, kk * S + tc_ * 128 : kk * S + (tc_ + 1) * 128],
                        wvt[:, kk * PG : (kk + 1) * PG],
                        start=(kk == 0),
                        stop=(kk == kc - 1),
                    )
                vslice = vaug[:, tc_ * (PG + HPC) : (tc_ + 1) * (PG + HPC)]
                nc.vector.tensor_copy(
                    vslice.rearrange("p (h e) -> p h e", e=DH + 1)[:, :, 0:DH],
                    acc[:].rearrange("p (h e) -> p h e", h=HPC),
                )

            ctxn = {}
            for st in range(NST):
                for sb in range(NSB):
                    ctxn[(st, sb)] = ctxp.tile(
                        [128, PG], f16, tag="ctxn", name=f"ctxn_{st}_{sb}"
                    )

            ex_tiles = {}

            def scores_block(st, h):
                # two t-chunks share one 2-bank psum tile so a single wide
                # activation (1024 free) amortizes the per-instruction access
                # bubble on the Activation engine
                c, po = h // 2, (h % 2) * 64
                fo = c * S
                exs = []
                for tb in range(NT // 2):
                    sc_ps = scp.tile(
                        [128, 1024], f32, tag="ps", name=f"sc_{st}_{h}_{tb}"
                    )
                    for half in range(2):
                        tch = tb * 2 + half
                        nc.tensor.matmul(
                            sc_ps[:, half * 512 : half * 512 + 512],
                            kproj[po : po + 64, fo + tch * 128 : fo + (tch + 1) * 128],
                            qproj[po : po + 64, fo + st * 512 : fo + st * 512 + 512],
                            start=True,
                            stop=True,
                        )
                    ex = expp.tile(
                        [128, 1024], f16, tag="exp", name=f"ex_{st}_{h}_{tb}"
                    )
                    nc.scalar.activation(ex[:], sc_ps[:], Exp)
                    exs.append(ex)
                ex_tiles[(st, h)] = exs

            # pv psum slots: pvp; in st1 alternate with pap slots (projection
            # accumulators are done by then, and pap tiles are allocated
            # before these in pool-rotation order)
            pv_pool_of = {}
            for _h in range(HPC):
                pv_pool_of[(0, _h)] = pap if _h % 2 == 0 else pvp
                pv_pool_of[(1, _h)] = pvp

            _pool_tag = {"pvp": "pv", "pap": "pa", "oap": "oa"}

            def pv_block(st, h):
                pool = pv_pool_of[(st, h)]
                pv = pool.tile(
                    [128, 512], f32, tag=_pool_tag[pool.name], name=f"pv_{st}_{h}"
                )
                exs = ex_tiles.pop((st, h))
                # tch-outer so each ex tile is fully consumed (all 4 sb groups)
                # as soon as its vaug chunk lands -> frees exp-pool slots early.
                # PSUM start zeroes the whole 2KB zero-region lazily, so only
                # the very first matmul starts; the other sb groups begin from
                # the pending-zero state.
                for tch in range(NT):
                    for sb in range(NSB):
                        nc.tensor.matmul(
                            pv[:, sb * 128 : sb * 128 + 65],
                            exs[tch // 2][
                                :, (tch % 2) * 512 + sb * 128 : (tch % 2) * 512 + (sb + 1) * 128
                            ],
                            vaug[
                                :,
                                tch * (PG + HPC) + h * (DH + 1) : tch * (PG + HPC)
                                + (h + 1) * (DH + 1),
                            ],
                            start=(tch == 0 and sb == 0),
                            stop=(tch == NT - 1 and sb == NSB - 1),
                            skip_group_check=True,
                        )
                rec4 = recp.tile([128, 4], f32, tag="rec", name=f"rec_{st}_{h}")
                nc.vector.reciprocal(
                    rec4[:].rearrange("p (g e) -> p g e", e=1),
                    pv[:].rearrange("p (g e) -> p g e", e=128)[:, :, 64:65],
                )
                for sb in range(NSB):
                    nc.vector.tensor_scalar(
                        ctxn[(st, sb)][:, h * DH : (h + 1) * DH],
                        pv[:, sb * 128 : sb * 128 + DH],
                        rec4[:, sb : sb + 1],
                        None,
                        mybir.AluOpType.mult,
                    )
                if h % 2 == 1:
                    j = h // 2
                    for sb in range(NSB):
                        nc.sync.dma_start_transpose(
                            ctxT[
                                :,
                                j * S + st * 512 + sb * 128 : j * S
                                + st * 512
                                + (sb + 1) * 128,
                            ],
                            ctxn[(st, sb)][:, j * 128 : (j + 1) * 128],
                        )

            def outproj(st, sbs=None):
                for sb in sbs if sbs is not None else range(NSB):
                    osb = dvep.tile([128, D], f16, tag="osb", name=f"osb_{st}_{sb}")
                    for nt in range(2):
                        gi = sb * 2 + nt
                        # st1 tail: scores pool is drained by then, borrow its
                        # banks so more groups pre-accumulate in parallel
                        pool, tg = (pap, "pa")
                        if st == 1 and sb == 1:
                            # pvp slots free right after the tail transposes
                            pool, tg = (pvp, "pv")
                        elif st == 1 and sb == 2:
                            pool, tg = (scp, "ps")
                        acc = pool.tile(
                            [128, 512], f32, tag=tg, name=f"oacc_{st}_{sb}_{nt}"
                        )
                        for j in range(4):
                            nc.tensor.matmul(
                                acc[:],
                                ctxT[
                                    :,
                                    j * S + st * 512 + sb * 128 : j * S
                                    + st * 512
                                    + (sb + 1) * 128,
                                ],
                                wot[:, j * D + nt * 512 : j * D + nt * 512 + 512],
                                start=(j == 0),
                                stop=(j == 3),
                            )
                        # tail evictions split across DVE and the by-then-idle
                        # Act engine so the last groups drain in parallel
                        if st == 1 and nt == 1:
                            nc.scalar.copy(osb[:, nt * 512 : nt * 512 + 512], acc[:])
                        else:
                            nc.vector.tensor_copy(
                                osb[:, nt * 512 : nt * 512 + 512], acc[:]
                            )
                    nc.sync.dma_start(
                        out[st * 512 + sb * 128 : st * 512 + (sb + 1) * 128, :], osb[:]
                    )

            # q0/q1 pre-run while xk loads; k0 immediately at the xk gate so
            # the exp stream starts earliest. q2/q3 fold into the head stream.
            # PV deferred one head behind its scores so PE never head-of-line
            # blocks on the exp stream.
            # Emission order = program order: every consumer AFTER its
            # producers. kproj needs BOTH free-halves per pair before that
            # pair's scores (free dim = key positions); qproj's halves map to
            # the query split so st1 halves defer. pv blocks must follow the
            # v chunks they read; the scheduler still streams their matmuls
            # as each vaug chunk lands (subtile deps).
            proj_half(xqt, wqt, qproj, 0, 0, "q")
            proj_half(xkt, wkt, kproj, 0, 0, "k")
            proj_half(xkt, wkt, kproj, 0, 1, "k", pool=pvp, tg="pv")
            scores_block(0, 0)
            scores_block(0, 1)
            proj_half(xqt, wqt, qproj, 1, 0, "q")
            proj_half(xkt, wkt, kproj, 1, 0, "k")
            proj_half(xkt, wkt, kproj, 1, 1, "k")
            scores_block(0, 2)
            scores_block(0, 3)
            proj_half(xqt, wqt, qproj, 2, 0, "q")
            proj_half(xkt, wkt, kproj, 2, 0, "k")
            proj_half(xkt, wkt, kproj, 2, 1, "k")
            scores_block(0, 4)
            scores_block(0, 5)
            proj_half(xqt, wqt, qproj, 3, 0, "q")
            proj_half(xkt, wkt, kproj, 3, 0, "k")
            proj_half(xkt, wkt, kproj, 3, 1, "k")
            scores_block(0, 6)
            scores_block(0, 7)
            proj_half(xqt, wqt, qproj, 0, 1, "q")
            scores_block(1, 0)
            scores_block(1, 1)
            for tc_ in range(NT):
                v_proj_chunk(tc_)
            pv_block(0, 0)
            pv_block(0, 1)
            proj_half(xqt, wqt, qproj, 1, 1, "q")
            for _h in range(2, HPC):
                pv_block(0, _h)
            scores_block(1, 2)
            scores_block(1, 3)
            pv_block(1, 0)
            pv_block(1, 1)
            outproj(0, (0, 1))
            proj_half(xqt, wqt, qproj, 2, 1, "q")
            scores_block(1, 4)
            scores_block(1, 5)
            pv_block(1, 2)
            pv_block(1, 3)
            outproj(0, (2, 3))
            proj_half(xqt, wqt, qproj, 3, 1, "q")
            scores_block(1, 6)
            scores_block(1, 7)
            pv_block(1, 4)
            pv_block(1, 5)
            pv_block(1, 6)
            pv_block(1, 7)
            # emitted last = lowest priority: pure PE-gap fillers, scheduled
            # whenever their ctxT pair-chunks are ready
            outproj(1)

    nc.compile()
    return nc


def _get_program(kc=8):
    if kc not in _prog_cache:
        _prog_cache[kc] = _build_program(kc)
    return _prog_cache[kc]


def _pad_k(a, kc):
    """(1024, n) f16 -> (kc*128, n); row 1024 = bias slot (set by caller) when kc=9."""
    if kc == 8:
        return np.ascontiguousarray(a, np.float16)
    p = np.zeros((kc * 128, a.shape[1]), np.float16)
    p[:D] = a
    return p


def _np_reference(q, k, v, attn_mask, Wq, bq, Wk, bk, Wv, bv, Wo, bo):
    def split_heads(x):
        return x.reshape(S, B, H, DH).transpose(2, 1, 0, 3)

    qh = split_heads(q @ Wq.T + bq)
    kh = split_heads(k @ Wk.T + bk)
    vh = split_heads(v @ Wv.T + bv)
    scores = np.einsum("hbsd,hbtd->hbst", qh, kh) * ATT_SCALE + attn_mask
    m = scores.max(-1, keepdims=True)
    e = np.exp(scores - m)
    probs = e / e.sum(-1, keepdims=True)
    ctx = np.einsum("hbst,hbtd->hbsd", probs, vh)
    ctx = ctx.transpose(2, 1, 0, 3).reshape(S, B, D)
    return (ctx @ Wo.T + bo).astype(np.float32)


def kernel(q, k, v, attn_mask, Wq, bq, Wk, bk, Wv, bv, Wo, bo, _want_results=False, _trace=False):
    q, k, v = (np.asarray(x, np.float32) for x in (q, k, v))
    attn_mask = np.asarray(attn_mask, np.float32)
    Wq, bq, Wk, bk, Wv, bv, Wo, bo = (
        np.asarray(x, np.float32) for x in (Wq, bq, Wk, bk, Wv, bv, Wo, bo)
    )
    if attn_mask.any():
        return _np_reference(q, k, v, attn_mask, Wq, bq, Wk, bk, Wv, bv, Wo, bo)

    from concourse import bass_utils

    zero_bias = not (bq.any() or bk.any() or bv.any())
    kc = 8 if zero_bias else 9
    nc = _get_program(kc)

    xT = {}
    for b in range(B):
        for nm, t in (("q", q), ("k", k), ("v", v)):
            a = _pad_k(t[:, b, :].T.astype(np.float16), kc)
            if kc > 8:
                a[D] = 1.0  # bias row
            xT[(nm, b)] = a

    in_maps = []
    for cid in range(N_CORES):
        b, g = cid >> 1, cid & 1
        cols = slice(g * PG, (g + 1) * PG)
        wqT = _pad_k((Wq[cols].T * ATT_SCALE).astype(np.float16), kc)
        wkT = _pad_k(Wk[cols].T.astype(np.float16), kc)
        wvT = _pad_k(Wv[cols].T.astype(np.float16), kc)
        if kc > 8:
            wqT[D] = (bq[cols] * ATT_SCALE).astype(np.float16)
            wkT[D] = bk[cols].astype(np.float16)
            wvT[D] = bv[cols].astype(np.float16)
        woT = np.ascontiguousarray(Wo[:, cols].T.astype(np.float16))
        in_maps.append(
            {
                "xq": xT[("q", b)],
                "xk": xT[("k", b)],
                "xv": xT[("v", b)],
                "wq": wqT,
                "wk": wkT,
                "wv": wvT,
                "wo": woT,
                "ident": np.eye(128, dtype=np.float16),
            }
        )

    import tempfile

    kw = {}
    if _trace:
        kw = dict(trace=True, tmpdir=tempfile.mkdtemp(prefix="bass_trace_"))
    res = bass_utils.run_bass_kernel_spmd(nc, in_maps, core_ids=list(range(N_CORES)), **kw)
    out = np.empty((S, B, D), np.float32)
    for b in range(B):
        out[:, b, :] = (
            res.results[2 * b]["out"].astype(np.float32)
            + res.results[2 * b + 1]["out"].astype(np.float32)
            + bo
        )
    if _want_results:
        return out, res
    return out
